# revision 7
# baseline (speedup 1.0000x reference)
"""DeepAR (2-layer LSTM encoder/decoder + gaussian heads) on 8 Trainium2 cores.

Strategy: pure data-parallel over batch B=1024 -> 128 rows/core (= SBUF
partition count). Everything on-chip per core:
  - batch-major layout [128 batch partitions, features] for states/elementwise
  - matmuls: out[b, gates] = lhsT.T @ rhs with lhsT = transposed activations
    (x^T / h^T chunks, [K<=128, 128]) stationary and rhs = pre-transposed
    weight chunks [K, 2048] moving, fp32r (full fp32 storage, fast PE mode),
    N-chunks of 512 into PSUM banks.
  - encoder L0 x-half (K=32) packed 4-up into PE row-groups via tile_position:
    xT replicated at partitions {0,32,64,96}, weight chunk n at block-row n,
    so the four 512-col streams run concurrently (~3x).
  - h^T produced each step via 4 PE transposes into one PSUM tile + a single
    DVE copy-back (fp32->fp32r), minimizing PE<->DVE ping-pong.
  - LSTM cell elementwise on DVE (bias add from PSUM, products) + ScalarE
    (sigmoid/tanh), biases pre-replicated across partitions host-side.
  - x DMA prefetched 3 steps ahead; its PE transpose emitted at the tail of
    the previous-previous step so it never heads the PE queue waiting on DMA.
  - decoder weights/biases partially prefetched into free pool slots during
    the encoder tail to shrink the transition bubble.

Per step the PE emission order interleaves the two layers so each layer's
elementwise latency hides under the other layer's independent matmul half:
  L0-h MMs | tpose h1(prev) | [dec: heads] | L0-x MMs | L0 el |
  L1-h MMs | tpose h0 | L1-x MMs | L1 el | x-tpose(s+2)
"""

import numpy as np

import concourse.bass as bass
import concourse.mybir as mybir
import concourse.tile as tile
from concourse.bacc import Bacc
from concourse.bass_utils import run_bass_kernel_spmd

f32 = mybir.dt.float32
f32r = mybir.dt.float32r
AF = mybir.ActivationFunctionType
OP = mybir.AluOpType

B, T, D, H, K_OUT, TAU = 1024, 168, 32, 512, 8, 24
NCORES = 8
BC = B // NCORES          # 128 batch rows per core
G = 4 * H                 # 2048 gate width
NB = G // 512             # 4 psum n-chunks
HK = H // 128             # 4 k-chunks per hidden input

WPOOL_BUFS = 17           # 12 encoder chunks + 5 decoder-prefetch slots


def build_nc(t_enc=T, t_dec=TAU):
    nc = Bacc()

    x_d = nc.dram_tensor("x", [BC, t_enc, D], f32, kind="ExternalInput")
    w_d = {}
    w_d["e0"] = nc.dram_tensor("w_e0", [HK, 128, G], f32r, kind="ExternalInput")
    for nm in ("e1", "d0", "d1"):
        w_d[nm] = nc.dram_tensor(f"w_{nm}", [2 * HK, 128, G], f32r, kind="ExternalInput")
    wx_d = nc.dram_tensor("w_x", [128, 512], f32r, kind="ExternalInput")
    b_d = {nm: nc.dram_tensor(f"b_{nm}", [BC, G], f32, kind="ExternalInput")
           for nm in ("e0", "e1", "d0", "d1")}
    wh_d = nc.dram_tensor("w_head", [HK, 128, 2 * K_OUT], f32r, kind="ExternalInput")
    bh_d = nc.dram_tensor("b_head", [BC, 2 * K_OUT], f32, kind="ExternalInput")
    id_d = nc.dram_tensor("ident", [128, 128], f32r, kind="ExternalInput")
    mu_d = nc.dram_tensor("mu", [BC, t_dec, K_OUT], f32, kind="ExternalOutput")
    sg_d = nc.dram_tensor("sigma", [BC, t_dec, K_OUT], f32, kind="ExternalOutput")

    with tile.TileContext(nc) as tc:
        with (
            tc.tile_pool(name="consts", bufs=1) as consts,
            tc.tile_pool(name="wpool", bufs=WPOOL_BUFS) as wpool,
            tc.tile_pool(name="bpool", bufs=3) as bpool,
            tc.tile_pool(name="xring", bufs=4) as xring,
            tc.tile_pool(name="tmps", bufs=10) as tmps,
            tc.tile_pool(name="gps", bufs=6, space="PSUM") as gps,
            tc.tile_pool(name="tps", bufs=2, space="PSUM") as tps,
        ):
            # ---------- startup loads ----------
            ident = consts.tile([128, 128], f32r, tag="ident")
            nc.sync.dma_start(ident, id_d[:, :])

            w_x = consts.tile([128, 512], f32r, tag="w_x")
            nc.sync.dma_start(w_x, wx_d[:, :])

            bias = {}

            def load_bias(nm):
                bias[nm] = bpool.tile([BC, G], f32, tag="b", name=f"b_{nm}")
                nc.sync.dma_start(bias[nm], b_d[nm][:, :])

            load_bias("e0")
            load_bias("e1")

            w_head = consts.tile([128, HK, 2 * K_OUT], f32r, tag="w_head")
            nc.sync.dma_start(w_head, wh_d[:, :, :].rearrange("k p n -> p k n"))
            b_head = consts.tile([BC, 2 * K_OUT], f32, tag="b_head")
            nc.sync.dma_start(b_head, bh_d[:, :])

            w = {"d0": [None] * (2 * HK), "d1": [None] * (2 * HK)}

            def load_w_chunk(nm, k):
                wt = wpool.tile([128, G], f32r, tag="w", name=f"w_{nm}_{k}")
                nc.sync.dma_start(wt, w_d[nm][k, :, :])
                return wt

            def load_w(nm, nk):
                return [load_w_chunk(nm, k) for k in range(nk)]

            w["e0"] = load_w("e0", HK)
            w["e1"] = load_w("e1", 2 * HK)

            # ---------- persistent state ----------
            c_st = {}
            hT = {}
            h_tmp = {}
            z0 = consts.tile([128, HK, 128], f32, tag="z0")
            nc.vector.memset(z0, 0.0)
            for l in (0, 1):
                c_st[l] = consts.tile([BC, H], f32, tag=f"c{l}", name=f"c{l}")
                nc.vector.memset(c_st[l], 0.0)
                hT[l] = consts.tile([128, HK, 128], f32r, tag=f"hT{l}", name=f"hT{l}")
                nc.vector.tensor_copy(hT[l], z0)
                h_tmp[l] = consts.tile([BC, H], f32r, tag=f"h_tmp{l}", name=f"h_tmp{l}")

            mu_sb = consts.tile([BC, t_dec * K_OUT], f32, tag="mu_sb")
            zs_sb = consts.tile([BC, t_dec * K_OUT], f32, tag="zs_sb")
            sg_sb = consts.tile([BC, t_dec * K_OUT], f32, tag="sg_sb")

            # persistent xT ring (row-replicated transposed x); all 128 rows
            # are written each refresh, so no stale-byte zeroing needed beyond
            # startup
            XRING = 4
            xT_ring = []
            for j in range(XRING):
                xt = consts.tile([128, 128], f32r, tag=f"xT{j}", name=f"xT{j}")
                nc.vector.tensor_copy(xt, z0[:, 0, :])
                xT_ring.append(xt)

            xs_ring = [None] * XRING

            # ---------- helpers ----------
            def x_dma(t):
                xs = xring.tile([BC, D], f32, tag="xs")
                nc.sync.dma_start(xs, x_d[:, t, :])
                xs_ring[t % XRING] = xs

            def x_tpose(t):
                """xs -> free-dim x4 replicate -> PE transpose -> xT ring.
                Transposed+replicated layout: row 32*r + d = x[:, d], so the
                four 32-row blocks feed four PE row-groups."""
                xf = xring.tile([128, 4, D], f32r, tag="xf")
                src = xs_ring[t % XRING][:, :].rearrange("p (o d) -> p o d", o=1)
                nc.vector.tensor_copy(xf, src.broadcast_to([128, 4, D]))
                tp = tps.tile([128, 512], f32r, tag="tp")
                nc.tensor.transpose(
                    tp[:, :128], xf.rearrange("p r d -> p (r d)"), ident)
                nc.vector.tensor_copy(xT_ring[t % XRING], tp[:, :128])

            def emit_mms(psums, w_chunks, lhsT_list, k_ids, start, stop):
                """Matmuls accumulating into psums[n]; n-outer frees PSUM
                banks early for the elementwise that follows."""
                for n in range(NB):
                    if psums[n] is None:
                        psums[n] = gps.tile([BC, 512], f32, tag="g", name=f"g{n}")
                for n in range(NB):
                    for j in range(len(k_ids)):
                        nc.tensor.matmul(
                            psums[n],
                            lhsT_list[j],
                            w_chunks[k_ids[j]][:, n * 512:(n + 1) * 512],
                            start=start and j == 0,
                            stop=stop and j == len(k_ids) - 1,
                        )
                return psums

            def emit_xmms(psums, xT):
                """Encoder L0 x-half: 4 concurrent K=32 row-group matmuls,
                one per psum n-chunk (tile_position row strips)."""
                for n in range(NB):
                    sl = slice(32 * n, 32 * (n + 1))
                    nc.tensor.matmul(
                        psums[n],
                        xT[sl, :],
                        w_x[sl, :],
                        start=False, stop=True,
                        tile_position=(32 * n, 0),
                    )
                return psums

            def emit_tpose(l):
                """h_tmp[l] -> hT[l]: 4 PE transposes into one PSUM tile,
                then a single DVE copy-back."""
                tp = tps.tile([128, 512], f32r, tag="tp")
                for k in range(HK):
                    nc.tensor.transpose(
                        tp[:, k * 128:(k + 1) * 128],
                        h_tmp[l][:, k * 128:(k + 1) * 128], ident)
                nc.vector.tensor_copy(
                    hT[l], tp.rearrange("p (k c) -> p k c", k=HK))

            def emit_el(l, psums, b_t):
                """LSTM cell elementwise: gates in psums (i,f,g,o), updates
                c_st[l] in place and writes h_tmp[l]."""
                zb = []
                for gidx in range(4):
                    z = tmps.tile([BC, 512], f32, tag="e")
                    nc.vector.tensor_tensor(
                        z, psums[gidx], b_t[:, gidx * 512:(gidx + 1) * 512], OP.add)
                    zb.append(z)
                si = tmps.tile([BC, 512], f32, tag="e")
                nc.scalar.activation(si, zb[0], AF.Sigmoid)
                sf = tmps.tile([BC, 512], f32, tag="e")
                nc.scalar.activation(sf, zb[1], AF.Sigmoid)
                tg = tmps.tile([BC, 512], f32, tag="e")
                nc.scalar.activation(tg, zb[2], AF.Tanh)
                so = tmps.tile([BC, 512], f32, tag="e")
                nc.scalar.activation(so, zb[3], AF.Sigmoid)
                t2 = tmps.tile([BC, 512], f32, tag="e")
                nc.vector.tensor_tensor(t2, si, tg, OP.mult)
                nc.vector.tensor_tensor(c_st[l], c_st[l], sf, OP.mult)
                nc.vector.tensor_tensor(c_st[l], c_st[l], t2, OP.add)
                tc_ = tmps.tile([BC, 512], f32, tag="e")
                nc.scalar.activation(tc_, c_st[l], AF.Tanh)
                nc.vector.tensor_tensor(h_tmp[l], so, tc_, OP.mult)

            def emit_heads(ti):
                """mu/sigma for decoder output index ti from hT[1]."""
                hp = tps.tile([128, 512], f32, tag="tp")
                for k in range(HK):
                    nc.tensor.matmul(
                        hp[:, :2 * K_OUT], hT[1][:, k, :], w_head[:, k, :],
                        start=(k == 0), stop=(k == HK - 1))
                sl = slice(ti * K_OUT, (ti + 1) * K_OUT)
                nc.vector.tensor_tensor(
                    mu_sb[:, sl], hp[:, :K_OUT], b_head[:, :K_OUT], OP.add)
                nc.vector.tensor_tensor(
                    zs_sb[:, sl], hp[:, K_OUT:2 * K_OUT],
                    b_head[:, K_OUT:2 * K_OUT], OP.add)

            # ---------- main loop (encoder then decoder, unified body) ----------
            for t in range(min(3, t_enc)):
                x_dma(t)
            x_tpose(0)
            if t_enc > 1:
                x_tpose(1)

            # decoder chunk prefetch order: consumption order at tau=0
            dec_prefetch = [("d0", 4), ("d0", 5), ("d0", 6), ("d0", 7), ("d0", 0)]
            pre_start = t_enc - len(dec_prefetch) - 2
            if pre_start < 4:  # tiny dev models: no prefetch
                dec_prefetch = []
                pre_start = -100

            for step in range(t_enc + t_dec):
                enc = step < t_enc
                tau = step - t_enc

                if enc and step + 3 < t_enc:
                    x_dma(step + 3)
                if enc and pre_start <= step < pre_start + len(dec_prefetch):
                    nm, k = dec_prefetch[step - pre_start]
                    w[nm][k] = load_w_chunk(nm, k)
                if enc and step == pre_start + len(dec_prefetch):
                    load_bias("d0")
                if not enc and tau == 0:
                    # remaining decoder weights/biases, in consumption order
                    for k in (4, 5, 6, 7, 0, 1, 2, 3):
                        if w["d0"][k] is None:
                            w["d0"][k] = load_w_chunk("d0", k)
                    for k in (4, 5, 6, 7, 0, 1, 2, 3):
                        w["d1"][k] = load_w_chunk("d1", k)
                    load_bias("d1")
                wl0, wl1 = (w["e0"], w["e1"]) if enc else (w["d0"], w["d1"])
                bl0, bl1 = (bias["e0"], bias["e1"]) if enc else (bias["d0"], bias["d1"])

                # 1. L0 h-half
                h_ids = list(range(HK)) if enc else list(range(HK, 2 * HK))
                psums0 = emit_mms([None] * NB, wl0,
                                  [hT[0][:, k, :] for k in range(HK)],
                                  h_ids, start=True, stop=False)
                # 2. transpose h1(prev)
                if step > 0:
                    emit_tpose(1)
                # 3. heads for previous decoder output
                if not enc and tau > 0:
                    emit_heads(tau - 1)
                # 4. L0 x-half
                if enc:
                    emit_xmms(psums0, xT_ring[step % XRING])
                else:
                    emit_mms(psums0, wl0, [hT[1][:, k, :] for k in range(HK)],
                             list(range(HK)), start=False, stop=True)
                # 5. L0 elementwise
                emit_el(0, psums0, bl0)
                # 6. L1 h-half
                psums1 = emit_mms([None] * NB, wl1,
                                  [hT[1][:, k, :] for k in range(HK)],
                                  list(range(HK, 2 * HK)), start=True, stop=False)
                # 7. transpose h0 -> hT0
                emit_tpose(0)
                # 8. L1 x-half
                emit_mms(psums1, wl1, [hT[0][:, k, :] for k in range(HK)],
                         list(range(HK)), start=False, stop=True)
                # 9. L1 elementwise
                emit_el(1, psums1, bl1)
                # 10. x transpose for step+2 (PE-queue tail; DMA long landed)
                if enc and step + 2 < t_enc:
                    x_tpose(step + 2)

            # final decoder output
            emit_tpose(1)
            emit_heads(t_dec - 1)

            # sigma = softplus(2z)/2 = ln(1 + exp(2z))/2 (no softplus table
            # on ACT; exp/ln share one table set, loaded once here)
            et = tmps.tile([BC, t_dec * K_OUT], f32, tag="e2")
            nc.scalar.activation(et, zs_sb, AF.Exp, scale=2.0)
            nc.scalar.activation(sg_sb, et, AF.Ln, bias=1.0)
            nc.vector.tensor_scalar_mul(sg_sb, sg_sb, 0.5)
            nc.sync.dma_start(
                mu_d[:, :, :], mu_sb.rearrange("b (t k) -> b t k", k=K_OUT))
            nc.sync.dma_start(
                sg_d[:, :, :], sg_sb.rearrange("b (t k) -> b t k", k=K_OUT))

    nc.finalize()
    return nc


def prep_weights(inp, t_enc=T):
    """Host-side weight layout prep. Returns the shared (non-x) input map."""
    def hcat(whh):
        return np.ascontiguousarray(whh.T.astype(np.float32)).reshape(HK, 128, G)

    def wcat(wih, whh):
        # K-space rows: [x-input dims (padded to HK*128), h dims]
        din = wih.shape[1]
        xpart = np.zeros((HK * 128, G), np.float32)
        xpart[:din] = wih.T
        return np.concatenate([xpart, whh.T.astype(np.float32)], axis=0) \
            .reshape(2 * HK, 128, G)

    m = {}
    m["w_e0"] = hcat(inp["enc_Whh0"])
    m["w_e1"] = wcat(inp["enc_Wih1"], inp["enc_Whh1"])
    m["w_d0"] = wcat(inp["dec_Wih0"], inp["dec_Whh0"])
    m["w_d1"] = wcat(inp["dec_Wih1"], inp["dec_Whh1"])
    # packed x weights: block-row n = Wih0.T slice for gate cols [512n, 512n+512)
    wx = np.zeros((128, 512), np.float32)
    wt = inp["enc_Wih0"].T.astype(np.float32)  # [D, G]
    for n in range(NB):
        wx[32 * n:32 * n + D] = wt[:, 512 * n:512 * (n + 1)]
    m["w_x"] = np.ascontiguousarray(wx)
    for nm, pre in (("e0", "enc_"), ("e1", "enc_"), ("d0", "dec_"), ("d1", "dec_")):
        i = nm[1]
        bsum = (inp[f"{pre}bih{i}"] + inp[f"{pre}bhh{i}"]).astype(np.float32)
        m[f"b_{nm}"] = np.ascontiguousarray(np.broadcast_to(bsum, (BC, G)))
    wh = np.concatenate([inp["W1"].T, inp["W2"].T], axis=1).astype(np.float32)  # [H, 16]
    m["w_head"] = np.ascontiguousarray(wh.reshape(HK, 128, 2 * K_OUT))
    bh = np.concatenate([inp["b1"], inp["b2"]]).astype(np.float32)
    m["b_head"] = np.ascontiguousarray(np.broadcast_to(bh, (BC, 2 * K_OUT)))
    m["ident"] = np.eye(128, dtype=np.float32)
    return m


_NC_CACHE = {}


def get_nc(t_enc=T, t_dec=TAU):
    key = (t_enc, t_dec)
    if key not in _NC_CACHE:
        _NC_CACHE[key] = build_nc(t_enc, t_dec)
    return _NC_CACHE[key]


def kernel(**inputs):
    inputs = {k: np.asarray(v) for k, v in inputs.items()}
    nc = get_nc()
    base = prep_weights(inputs)
    x = inputs["x"].astype(np.float32)
    in_maps = [dict(base, x=np.ascontiguousarray(x[i * BC:(i + 1) * BC]))
               for i in range(NCORES)]
    res = run_bass_kernel_spmd(nc, in_maps, core_ids=list(range(NCORES)))
    mu = np.concatenate([r["mu"] for r in res.results], axis=0)
    sigma = np.concatenate([r["sigma"] for r in res.results], axis=0)
    return mu, sigma
